# revision 12
# baseline (speedup 1.0000x reference)
"""BiLSTM-CRF Viterbi decode kernel for 8 Trainium2 NeuronCores.

Problem shapes (hardcoded): V=50257, E=128, H=128, T=12, B=64, S=512.

Sharding: data-parallel over batch, 8 sequences per core. Each core runs
the forward and backward LSTM scans interleaved (independent chains keep
all engines busy), computes emissions, and runs the CRF Viterbi forward
scan, emitting the per-step score series.  The host does constant prep
(bias folding) and the integer backtrace from the score series.
"""

import numpy as np

V, E, H, T, B, S = 50257, 128, 128, 12, 64, 512
NCORES = 8
PB = B // NCORES          # batch per core = 8
NBLK = (S * PB) // 128    # 128-token gather/matmul blocks = 32
G4 = 4 * H                # 512 gate rows
# gate order used on device: i, f, o, g  (PyTorch order is i, f, g, o)
GATE_PERM = [0, 1, 3, 2]

_PROGRAM_CACHE = {}
LAST_RESULT = None
DT_MM = "f32"


def _np_dt(dt_mm):
    import ml_dtypes
    return {"f32": np.float32, "f16": np.float16,
            "bf16": ml_dtypes.bfloat16}[dt_mm]


def build_program(s_len=S, pb=PB, dt_mm="f32", dt_xg="f16"):
    """Build the Bass/Tile SPMD program for one core. Returns (nc, names)."""
    import concourse.bacc as bacc
    import concourse.bass as bass
    import concourse.mybir as mybir
    import concourse.tile as tile

    fp32 = mybir.dt.float32
    DTMM = {"f32": mybir.dt.float32, "f16": mybir.dt.float16,
            "bf16": mybir.dt.bfloat16}[dt_mm]
    DTXG = {"f32": mybir.dt.float32, "f16": mybir.dt.float16,
            "bf16": mybir.dt.bfloat16}[dt_xg]
    AF = mybir.ActivationFunctionType
    ALU = mybir.AluOpType
    AX = mybir.AxisListType

    nblk = (s_len * pb) // 128
    ntok = s_len * pb

    nc = bacc.Bacc("TRN2", target_bir_lowering=False, debug=False)

    # ---- DRAM I/O ----
    d_emb = nc.dram_tensor("emb_w", [V, E], DTMM, kind="ExternalInput")
    d_ids = nc.dram_tensor("ids", [128, nblk], mybir.dt.int32,
                           kind="ExternalInput")
    d_wih = {}
    d_whh = {}
    d_bm = {}
    for d in ("f", "b"):
        d_wih[d] = nc.dram_tensor(f"wih_{d}", [E, G4], DTMM,
                                  kind="ExternalInput")
        d_whh[d] = nc.dram_tensor(f"whh_{d}", [H, G4], DTMM,
                                  kind="ExternalInput")
        d_bm[d] = nc.dram_tensor(f"biasmat_{d}", [4, 128], DTMM,
                                 kind="ExternalInput")
    d_ind = nc.dram_tensor("bias_ind", [4, 4 * 128], DTMM,
                           kind="ExternalInput")
    d_wof = nc.dram_tensor("wout_f", [H, T], DTMM, kind="ExternalInput")
    d_wob = nc.dram_tensor("wout_b", [H, T], DTMM, kind="ExternalInput")
    d_ident = nc.dram_tensor("ident", [128, 128], DTMM, kind="ExternalInput")
    d_start = nc.dram_tensor("start_t", [pb, T], fp32, kind="ExternalInput")
    d_trep = nc.dram_tensor("transrep", [pb, T * T], fp32,
                            kind="ExternalInput")
    d_scores = nc.dram_tensor("scores", [pb, s_len, T], fp32,
                              kind="ExternalOutput")

    with tile.TileContext(nc) as tc:
        with (
            tc.tile_pool(name="singles", bufs=1) as singles,
            tc.tile_pool(name="big", bufs=1) as big,
            tc.tile_pool(name="crf", bufs=2) as crf,
        ):
            # ---- load constants ----
            sb_wih = {}
            sb_whh = {}
            sb_bm = {}
            for d in ("f", "b"):
                sb_wih[d] = singles.tile([E, G4], DTMM, tag=f"wih{d}", name=f"wih{d}")
                nc.sync.dma_start(out=sb_wih[d][:], in_=d_wih[d].ap())
                sb_whh[d] = singles.tile([H, G4], DTMM, tag=f"whh{d}", name=f"whh{d}")
                nc.sync.dma_start(out=sb_whh[d][:], in_=d_whh[d].ap())
                sb_bm[d] = singles.tile([4, 128], DTMM, tag=f"bm{d}", name=f"bm{d}")
                nc.sync.dma_start(out=sb_bm[d][:], in_=d_bm[d].ap())
            sb_ind = singles.tile([4, 4 * 128], DTMM, tag="ind", name="ind")
            nc.sync.dma_start(out=sb_ind[:], in_=d_ind.ap())
            sb_wof = singles.tile([H, T], DTMM, tag="wof", name="wof")
            nc.sync.dma_start(out=sb_wof[:], in_=d_wof.ap())
            sb_wob = singles.tile([H, T], DTMM, tag="wob", name="wob")
            nc.sync.dma_start(out=sb_wob[:], in_=d_wob.ap())
            sb_ident = singles.tile([128, 128], DTMM, tag="ident", name="ident")
            nc.sync.dma_start(out=sb_ident[:], in_=d_ident.ap())
            sb_start = singles.tile([pb, T], fp32, tag="start", name="start")
            nc.sync.dma_start(out=sb_start[:], in_=d_start.ap())
            sb_trep = singles.tile([pb, T * T], fp32, tag="trep", name="trep")
            nc.sync.dma_start(out=sb_trep[:], in_=d_trep.ap())
            sb_ids = singles.tile([128, nblk], mybir.dt.int32, tag="ids", name="ids")
            nc.sync.dma_start(out=sb_ids[:], in_=d_ids.ap())

            # ---- persistent big buffers ----
            xg = {d: big.tile([128, 4, ntok], DTXG, tag=f"xg{d}", name=f"xg{d}")
                  for d in ("f", "b")}
            hT = {d: big.tile([128, s_len, pb], DTMM, tag=f"hT{d}", name=f"hT{d}")
                  for d in ("f", "b")}
            em_sb = big.tile([pb, s_len, T], fp32, tag="emsb", name="emsb")
            score = big.tile([pb, s_len, T], fp32, tag="score", name="score")

            # ---- phase 1+2: gather embeddings, transpose to [E, tok] ----
            with (
                tc.tile_pool(name="gather", bufs=3) as gather,
                tc.tile_pool(name="tps", bufs=2, space="PSUM") as tps,
                tc.tile_pool(name="xt", bufs=1) as xtp,
                tc.tile_pool(name="xgps", bufs=2, space="PSUM") as xgps,
            ):
                xT = xtp.tile([128, nblk, 128], DTMM, tag="xT", name="xT")
                for k in range(nblk):
                    ge = gather.tile([128, E], DTMM, tag="ge", name="ge")
                    nc.gpsimd.indirect_dma_start(
                        out=ge[:],
                        out_offset=None,
                        in_=d_emb.ap(),
                        in_offset=bass.IndirectOffsetOnAxis(
                            ap=sb_ids[:, k:k + 1], axis=0),
                    )
                    pt = tps.tile([128, 128], DTMM, tag="pt", name="pt")
                    nc.tensor.transpose(out=pt[:], in_=ge[:],
                                        identity=sb_ident[:])
                    if k % 2 == 0:
                        nc.vector.tensor_copy(xT[:, k, :], pt[:])
                    else:
                        nc.scalar.copy(xT[:, k, :], pt[:])

                # ---- phase 3: input gates xg = Wih @ x + bias ----
                for k in range(nblk):
                    for d in ("f", "b"):
                        ps = xgps.tile([128, 4, 128], fp32, tag="xgps", name="xgps")
                        nc.tensor.matmul(
                            ps[:].rearrange("p a b -> p (a b)"),
                            sb_bm[d][:],
                            sb_ind[:],
                            start=True, stop=False,
                            skip_group_check=True,
                        )
                        for j in range(4):
                            nc.tensor.matmul(
                                ps[:, j, :],
                                sb_wih[d][:, j * 128:(j + 1) * 128],
                                xT[:, k, :],
                                start=False, stop=(j == 3),
                                skip_group_check=True,
                            )
                        dst = xg[d][:, :, k * 128:(k + 1) * 128]
                        if k % 2 == 0:
                            nc.scalar.copy(dst, ps[:])
                        else:
                            nc.vector.tensor_copy(dst, ps[:])

            # ---- phase 4: LSTM scans (fwd + bwd interleaved) ----
            with (
                tc.tile_pool(name="gps_f", bufs=2, space="PSUM") as gps_f,
                tc.tile_pool(name="gps_b", bufs=2, space="PSUM") as gps_b,
                tc.tile_pool(name="state", bufs=1) as state,
                tc.tile_pool(name="step", bufs=3) as step,
            ):
                c_tiles = {d: [state.tile([128, pb], fp32, tag=f"c{d}{i}", name=f"c{d}{i}")
                               for i in range(2)] for d in ("f", "b")}
                zero_h = state.tile([128, pb], DTMM, tag="zeroh", name="zeroh")
                nc.vector.memset(zero_h[:], 0.0)

                for t in range(s_len):
                    for d in ("f", "b"):
                        tk = t if d == "f" else s_len - 1 - t
                        gp = (gps_f if d == "f" else gps_b)
                        c_prev = c_tiles[d][(t + 1) % 2]
                        c_new = c_tiles[d][t % 2]

                        gs = step.tile([128, 4 * pb], fp32, tag=f"gs{d}", name=f"gs{d}")
                        gs3 = gs[:].rearrange("p (a b) -> p a b", a=4)
                        if t > 0:
                            tk_prev = tk - 1 if d == "f" else tk + 1
                            ps = gp.tile([128, 4, pb], fp32, tag=f"g{d}", name=f"g{d}")
                            h_prev = hT[d][:, tk_prev, :]
                            for j in range(4):
                                nc.tensor.matmul(
                                    ps[:, j, :],
                                    sb_whh[d][:, j * 128:(j + 1) * 128],
                                    h_prev,
                                    start=True, stop=True,
                                    skip_group_check=True,
                                )
                            nc.vector.tensor_add(
                                gs3,
                                ps[:],
                                xg[d][:, :, tk * pb:(tk + 1) * pb],
                            )
                        else:
                            nc.vector.tensor_copy(
                                gs3,
                                xg[d][:, :, tk * pb:(tk + 1) * pb],
                            )

                        a_ifo = step.tile([128, 3 * pb], fp32, tag=f"a{d}", name=f"a{d}")
                        nc.scalar.activation(a_ifo[:], gs[:, 0:3 * pb],
                                             AF.Sigmoid)
                        t_g = step.tile([128, pb], fp32, tag=f"tg{d}", name=f"tg{d}")
                        nc.scalar.activation(t_g[:], gs[:, 3 * pb:4 * pb],
                                             AF.Tanh)

                        m2 = step.tile([128, pb], fp32, tag=f"m2{d}", name=f"m2{d}")
                        nc.vector.tensor_mul(m2[:], a_ifo[:, 0:pb], t_g[:])
                        if t > 0:
                            m1 = step.tile([128, pb], fp32, tag=f"m1{d}", name=f"m1{d}")
                            nc.vector.tensor_mul(m1[:], a_ifo[:, pb:2 * pb],
                                                 c_prev[:])
                            nc.vector.tensor_add(c_new[:], m1[:], m2[:])
                        else:
                            nc.vector.tensor_copy(c_new[:], m2[:])

                        t_c = step.tile([128, pb], fp32, tag=f"tc{d}", name=f"tc{d}")
                        nc.scalar.activation(t_c[:], c_new[:], AF.Tanh)
                        nc.vector.tensor_mul(hT[d][:, tk, :],
                                             a_ifo[:, 2 * pb:3 * pb], t_c[:])

            # ---- phase 5: emissions into PSUM [(s16,b), blk, T] ----
            from contextlib import ExitStack
            _emctx = ExitStack()
            empool = _emctx.enter_context(
                tc.tile_pool(name="empool", bufs=1, space="PSUM"))
            em_ps = empool.tile([128, nblk, T], fp32, tag="emps", name="emps")
            for k in range(nblk):
                nc.tensor.matmul(
                    em_ps[:, k, :],
                    hT["f"][:].rearrange("p s b -> p (s b)")
                    [:, k * 128:(k + 1) * 128],
                    sb_wof[:],
                    start=True, stop=False, skip_group_check=True,
                )
                nc.tensor.matmul(
                    em_ps[:, k, :],
                    hT["b"][:].rearrange("p s b -> p (s b)")
                    [:, k * 128:(k + 1) * 128],
                    sb_wob[:],
                    start=False, stop=True, skip_group_check=True,
                )

            # stage PSUM -> SBUF, then reshuffle [(s16,b) part, blk, T]
            # -> em_sb [b part, s, T] with per-b DMAs
            em_stage = big.tile([128, nblk, T], fp32, tag="emstage",
                                name="emstage")
            half = (nblk // 2) * T
            nc.vector.tensor_copy(
                em_stage[:].rearrange("p a b -> p (a b)")[:, 0:half],
                em_ps[:].rearrange("p a b -> p (a b)")[:, 0:half])
            nc.scalar.copy(
                em_stage[:].rearrange("p a b -> p (a b)")[:, half:nblk * T],
                em_ps[:].rearrange("p a b -> p (a b)")[:, half:nblk * T])
            s16cnt = 128 // pb
            _emctx.close()
            pitch = nblk * T
            for s16 in range(s16cnt):
                src_ap = bass.AP(
                    em_stage[:].tensor,
                    em_stage[:].offset + s16 * pb * pitch,
                    [[pitch, pb], [T, nblk], [1, T]],
                )
                dst_ap = bass.AP(
                    em_sb[:].tensor,
                    em_sb[:].offset + s16 * T,
                    [[s_len * T, pb], [s16cnt * T, nblk], [1, T]],
                )
                nc.sync.dma_start(out=dst_ap, in_=src_ap)

            # ---- phase 6: CRF Viterbi forward ----
            nc.vector.tensor_add(score[:, 0, :], em_sb[:, 0, :],
                                 sb_start[:])
            for t in range(1, s_len):
                tmp = crf.tile([pb, T * T], fp32, tag="tmp", name="tmp")
                prev = score[:, t - 1, :].rearrange(
                    "p (o c) -> p o c", o=1).to_broadcast([pb, T, T])
                nc.vector.tensor_tensor(
                    out=tmp[:], in0=prev,
                    in1=sb_trep[:], op=ALU.add)
                mx = crf.tile([pb, T], fp32, tag="mx", name="mx")
                nc.vector.tensor_reduce(
                    out=mx[:],
                    in_=tmp[:].rearrange("p (c q) -> p c q", q=T),
                    axis=AX.X, op=ALU.max)
                nc.vector.tensor_add(score[:, t, :], mx[:],
                                     em_sb[:, t, :])

            nc.sync.dma_start(out=d_scores.ap(), in_=score[:])

    nc.compile()
    return nc


def _prep_host(inputs, dt_np):
    """Build per-core in_maps from full inputs."""
    x = np.asarray(inputs["x"])
    emb = np.asarray(inputs["emb"], dtype=np.float32)
    w_out = np.asarray(inputs["w_out"], dtype=np.float32)
    b_out = np.asarray(inputs["b_out"], dtype=np.float32)
    start = np.asarray(inputs["start"], dtype=np.float32)
    trans = np.asarray(inputs["trans"], dtype=np.float32)

    def perm_rows(w):
        chunks = [w[i * H:(i + 1) * H] for i in range(4)]
        return np.concatenate([chunks[i] for i in GATE_PERM], axis=0)

    shared = {"emb_w": emb.astype(dt_np)}
    for d, (wi, wh, bb) in (("f", ("w_ih_f", "w_hh_f", "b_f")),
                            ("b", ("w_ih_b", "w_hh_b", "b_b"))):
        wih = perm_rows(np.asarray(inputs[wi], dtype=np.float32))
        whh = perm_rows(np.asarray(inputs[wh], dtype=np.float32))
        bias = perm_rows(np.asarray(inputs[bb],
                                    dtype=np.float32).reshape(-1, 1))[:, 0]
        shared[f"wih_{d}"] = np.ascontiguousarray(wih.T).astype(dt_np)
        shared[f"whh_{d}"] = np.ascontiguousarray(whh.T).astype(dt_np)
        shared[f"biasmat_{d}"] = bias.reshape(4, 128).astype(dt_np)
    ind = np.zeros((4, 4, 128), dtype=np.float32)
    for j in range(4):
        ind[j, j, :] = 1.0
    shared["bias_ind"] = ind.reshape(4, 512).astype(dt_np)
    shared["wout_f"] = np.ascontiguousarray(w_out[:, :H].T).astype(dt_np)
    shared["wout_b"] = np.ascontiguousarray(w_out[:, H:].T).astype(dt_np)
    shared["ident"] = np.eye(128, dtype=np.float32).astype(dt_np)
    shared["start_t"] = np.tile((start + b_out)[None, :], (PB, 1)).astype(
        np.float32)
    trep = (trans + b_out[None, :]).T.reshape(-1)  # [(c,p)]
    shared["transrep"] = np.tile(trep[None, :], (PB, 1)).astype(np.float32)

    in_maps = []
    for k in range(NCORES):
        xc = x[k * PB:(k + 1) * PB]              # [pb, S]
        ids = np.ascontiguousarray(
            xc.T.reshape(-1).reshape(NBLK, 128).T).astype(np.int32)
        m = dict(shared)
        m["ids"] = ids
        in_maps.append(m)
    return in_maps


def _host_finalize(scores, trans, end):
    """scores [B, S, T] f32 -> (path int32 [B,S], best f32 [B])."""
    final = scores[:, -1, :] + end[None, :]
    last = np.argmax(final, axis=-1).astype(np.int32)
    best = final.max(axis=-1).astype(np.float32)
    bidx = np.arange(scores.shape[0])
    path = np.empty((scores.shape[0], scores.shape[1]), dtype=np.int32)
    path[:, -1] = last
    tag = last
    for t in range(scores.shape[1] - 1, 0, -1):
        val = scores[:, t - 1, :] + trans[:, tag].T   # [B, T(prev)]
        tag = np.argmax(val, axis=-1).astype(np.int32)
        path[:, t - 1] = tag
    return path, best


def _reference_np(inputs):
    """Exact numpy fallback (general mask)."""
    x = np.asarray(inputs["x"])
    mask = np.asarray(inputs["mask"])
    emb = np.asarray(inputs["emb"], np.float32)
    xt = emb[x].transpose(1, 0, 2)

    def lstm(xg, whh):
        h = np.zeros((xg.shape[1], whh.shape[1]), np.float32)
        c = np.zeros_like(h)
        hs = []
        for g_t in xg:
            g = g_t + h @ whh.T
            i, f, gg, o = np.split(g, 4, -1)
            sig = lambda z: 1.0 / (1.0 + np.exp(-z))
            c = sig(f) * c + sig(i) * np.tanh(gg)
            h = sig(o) * np.tanh(c)
            hs.append(h)
        return np.stack(hs)

    xg_f = xt @ np.asarray(inputs["w_ih_f"], np.float32).T + np.asarray(
        inputs["b_f"], np.float32)
    xg_b = xt[::-1] @ np.asarray(inputs["w_ih_b"], np.float32).T + np.asarray(
        inputs["b_b"], np.float32)
    h = np.concatenate([lstm(xg_f, np.asarray(inputs["w_hh_f"], np.float32)),
                        lstm(xg_b, np.asarray(inputs["w_hh_b"],
                                              np.float32))[::-1]], -1)
    em = h @ np.asarray(inputs["w_out"], np.float32).T + np.asarray(
        inputs["b_out"], np.float32)
    trans = np.asarray(inputs["trans"], np.float32)
    m = mask.T
    sc = np.asarray(inputs["start"], np.float32) + em[0]
    hist = []
    for t in range(1, em.shape[0]):
        tot = sc[:, :, None] + trans[None]
        best, idx = tot.max(1), tot.argmax(1).astype(np.int32)
        hist.append(idx)
        sc = np.where(m[t][:, None], best + em[t], sc)
    final = sc + np.asarray(inputs["end"], np.float32)
    last = np.argmax(final, -1).astype(np.int32)
    best = final.max(-1)
    tags = [last]
    tag = last
    bidx = np.arange(x.shape[0])
    for t in range(em.shape[0] - 2, -1, -1):
        prev = hist[t][bidx, tag]
        tag = np.where(m[t + 1], prev, tag)
        tags.append(tag)
    path = np.stack(tags[::-1], 1).astype(np.int32)
    return path, best.astype(np.float32)


def kernel(**inputs):
    mask = np.asarray(inputs["mask"])
    if not mask.all():
        return _reference_np(inputs)

    dt_mm = DT_MM
    key = (S, PB, dt_mm, "f16")
    if key not in _PROGRAM_CACHE:
        _PROGRAM_CACHE[key] = build_program(S, PB, dt_mm, "f16")
    nc = _PROGRAM_CACHE[key]

    from concourse.bass_utils import run_bass_kernel_spmd
    in_maps = _prep_host(inputs, _np_dt(dt_mm))
    res = run_bass_kernel_spmd(nc, in_maps, core_ids=list(range(NCORES)))
    global LAST_RESULT
    LAST_RESULT = res
    scores = np.concatenate([res.results[k]["scores"]
                             for k in range(NCORES)], axis=0)
    trans = np.asarray(inputs["trans"], np.float32)
    end = np.asarray(inputs["end"], np.float32)
    path, best = _host_finalize(scores, trans, end)
    return path, best


# revision 13
# speedup vs baseline: 2.0985x; 2.0985x over previous
"""BiLSTM-CRF Viterbi decode kernel for 8 Trainium2 NeuronCores.

Problem shapes (hardcoded): V=50257, E=128, H=128, T=12, B=64, S=512.

Sharding: data-parallel over batch, 8 sequences per core. Each core runs
the forward and backward LSTM scans interleaved (independent chains keep
all engines busy), computes emissions, and runs the CRF Viterbi forward
scan, emitting the per-step score series.  The host does constant prep
(bias folding) and the integer backtrace from the score series.
"""

import numpy as np

V, E, H, T, B, S = 50257, 128, 128, 12, 64, 512
NCORES = 8
PB = B // NCORES          # batch per core = 8
NBLK = (S * PB) // 128    # 128-token gather/matmul blocks = 32
G4 = 4 * H                # 512 gate rows
# gate order used on device: i, f, o, g  (PyTorch order is i, f, g, o)
GATE_PERM = [0, 1, 3, 2]

_PROGRAM_CACHE = {}
LAST_RESULT = None
import os
DT_MM = os.environ.get("KDT", "f16")


def _np_dt(dt_mm):
    import ml_dtypes
    return {"f32": np.float32, "f16": np.float16,
            "bf16": ml_dtypes.bfloat16}[dt_mm]


def build_program(s_len=S, pb=PB, dt_mm="f32", dt_xg="f16"):
    """Build the Bass/Tile SPMD program for one core. Returns (nc, names)."""
    import concourse.bacc as bacc
    import concourse.bass as bass
    import concourse.mybir as mybir
    import concourse.tile as tile

    fp32 = mybir.dt.float32
    DTMM = {"f32": mybir.dt.float32, "f16": mybir.dt.float16,
            "bf16": mybir.dt.bfloat16}[dt_mm]
    DTXG = {"f32": mybir.dt.float32, "f16": mybir.dt.float16,
            "bf16": mybir.dt.bfloat16}[dt_xg]
    AF = mybir.ActivationFunctionType
    ALU = mybir.AluOpType
    AX = mybir.AxisListType

    nblk = (s_len * pb) // 128
    ntok = s_len * pb

    nc = bacc.Bacc("TRN2", target_bir_lowering=False, debug=False)

    # ---- DRAM I/O ----
    d_emb = nc.dram_tensor("emb_w", [V, E], DTMM, kind="ExternalInput")
    d_ids = nc.dram_tensor("ids", [128, nblk], mybir.dt.int32,
                           kind="ExternalInput")
    d_wih = {}
    d_whh = {}
    d_bm = {}
    for d in ("f", "b"):
        d_wih[d] = nc.dram_tensor(f"wih_{d}", [E, G4], DTMM,
                                  kind="ExternalInput")
        d_whh[d] = nc.dram_tensor(f"whh_{d}", [H, G4], DTMM,
                                  kind="ExternalInput")
        d_bm[d] = nc.dram_tensor(f"biasmat_{d}", [4, 128], DTMM,
                                 kind="ExternalInput")
    d_ind = nc.dram_tensor("bias_ind", [4, 4 * 128], DTMM,
                           kind="ExternalInput")
    d_wof = nc.dram_tensor("wout_f", [H, T], DTMM, kind="ExternalInput")
    d_wob = nc.dram_tensor("wout_b", [H, T], DTMM, kind="ExternalInput")
    d_ident = nc.dram_tensor("ident", [128, 128], DTMM, kind="ExternalInput")
    d_start = nc.dram_tensor("start_t", [pb, T], fp32, kind="ExternalInput")
    d_trep = nc.dram_tensor("transrep", [pb, T * T], fp32,
                            kind="ExternalInput")
    d_scores = nc.dram_tensor("scores", [pb, s_len, T], fp32,
                              kind="ExternalOutput")

    with tile.TileContext(nc) as tc:
        with (
            tc.tile_pool(name="singles", bufs=1) as singles,
            tc.tile_pool(name="big", bufs=1) as big,
            tc.tile_pool(name="crf", bufs=2) as crf,
        ):
            # ---- load constants ----
            sb_wih = {}
            sb_whh = {}
            sb_bm = {}
            for d in ("f", "b"):
                sb_wih[d] = singles.tile([E, G4], DTMM, tag=f"wih{d}", name=f"wih{d}")
                nc.sync.dma_start(out=sb_wih[d][:], in_=d_wih[d].ap())
                sb_whh[d] = singles.tile([H, G4], DTMM, tag=f"whh{d}", name=f"whh{d}")
                nc.sync.dma_start(out=sb_whh[d][:], in_=d_whh[d].ap())
                sb_bm[d] = singles.tile([4, 128], DTMM, tag=f"bm{d}", name=f"bm{d}")
                nc.sync.dma_start(out=sb_bm[d][:], in_=d_bm[d].ap())
            sb_ind = singles.tile([4, 4 * 128], DTMM, tag="ind", name="ind")
            nc.sync.dma_start(out=sb_ind[:], in_=d_ind.ap())
            sb_wof = singles.tile([H, T], DTMM, tag="wof", name="wof")
            nc.sync.dma_start(out=sb_wof[:], in_=d_wof.ap())
            sb_wob = singles.tile([H, T], DTMM, tag="wob", name="wob")
            nc.sync.dma_start(out=sb_wob[:], in_=d_wob.ap())
            sb_ident = singles.tile([128, 128], DTMM, tag="ident", name="ident")
            nc.sync.dma_start(out=sb_ident[:], in_=d_ident.ap())
            sb_start = singles.tile([pb, T], fp32, tag="start", name="start")
            nc.sync.dma_start(out=sb_start[:], in_=d_start.ap())
            sb_trep = singles.tile([pb, T * T], fp32, tag="trep", name="trep")
            nc.sync.dma_start(out=sb_trep[:], in_=d_trep.ap())
            sb_ids = singles.tile([128, nblk], mybir.dt.int32, tag="ids", name="ids")
            nc.sync.dma_start(out=sb_ids[:], in_=d_ids.ap())

            # ---- persistent big buffers ----
            xg = {d: big.tile([128, 4, ntok], DTXG, tag=f"xg{d}", name=f"xg{d}")
                  for d in ("f", "b")}
            hT = {d: big.tile([128, s_len, pb], DTMM, tag=f"hT{d}", name=f"hT{d}")
                  for d in ("f", "b")}
            em_sb = big.tile([pb, s_len, T], fp32, tag="emsb", name="emsb")
            score = big.tile([pb, s_len, T], fp32, tag="score", name="score")

            # ---- phase 1+2: gather embeddings, transpose to [E, tok] ----
            with (
                tc.tile_pool(name="gather", bufs=3) as gather,
                tc.tile_pool(name="tps", bufs=2, space="PSUM") as tps,
                tc.tile_pool(name="xt", bufs=1) as xtp,
                tc.tile_pool(name="xgps", bufs=2, space="PSUM") as xgps,
            ):
                xT = xtp.tile([128, nblk, 128], DTMM, tag="xT", name="xT")
                for k in range(nblk):
                    ge = gather.tile([128, E], DTMM, tag="ge", name="ge")
                    nc.gpsimd.indirect_dma_start(
                        out=ge[:],
                        out_offset=None,
                        in_=d_emb.ap(),
                        in_offset=bass.IndirectOffsetOnAxis(
                            ap=sb_ids[:, k:k + 1], axis=0),
                    )
                    pt = tps.tile([128, 128], DTMM, tag="pt", name="pt")
                    nc.tensor.transpose(out=pt[:], in_=ge[:],
                                        identity=sb_ident[:])
                    if k % 2 == 0:
                        nc.vector.tensor_copy(xT[:, k, :], pt[:])
                    else:
                        nc.scalar.copy(xT[:, k, :], pt[:])

                # ---- phase 3: input gates xg = Wih @ x + bias ----
                for k in range(nblk):
                    for d in ("f", "b"):
                        ps = xgps.tile([128, 4, 128], fp32, tag="xgps", name="xgps")
                        nc.tensor.matmul(
                            ps[:].rearrange("p a b -> p (a b)"),
                            sb_bm[d][:],
                            sb_ind[:],
                            start=True, stop=False,
                            skip_group_check=True,
                        )
                        for j in range(4):
                            nc.tensor.matmul(
                                ps[:, j, :],
                                sb_wih[d][:, j * 128:(j + 1) * 128],
                                xT[:, k, :],
                                start=False, stop=(j == 3),
                                skip_group_check=True,
                            )
                        dst = xg[d][:, :, k * 128:(k + 1) * 128]
                        if k % 2 == 0:
                            nc.scalar.copy(dst, ps[:])
                        else:
                            nc.vector.tensor_copy(dst, ps[:])

            # ---- phase 4: LSTM scans (fwd + bwd interleaved) ----
            with (
                tc.tile_pool(name="gps_f", bufs=2, space="PSUM") as gps_f,
                tc.tile_pool(name="gps_b", bufs=2, space="PSUM") as gps_b,
                tc.tile_pool(name="state", bufs=1) as state,
                tc.tile_pool(name="step", bufs=3) as step,
            ):
                c_tiles = {d: [state.tile([128, pb], fp32, tag=f"c{d}{i}", name=f"c{d}{i}")
                               for i in range(2)] for d in ("f", "b")}
                zero_h = state.tile([128, pb], DTMM, tag="zeroh", name="zeroh")
                nc.vector.memset(zero_h[:], 0.0)

                for t in range(s_len):
                    for d in ("f", "b"):
                        tk = t if d == "f" else s_len - 1 - t
                        gp = (gps_f if d == "f" else gps_b)
                        c_prev = c_tiles[d][(t + 1) % 2]
                        c_new = c_tiles[d][t % 2]

                        gs = step.tile([128, 4 * pb], fp32, tag=f"gs{d}", name=f"gs{d}")
                        gs3 = gs[:].rearrange("p (a b) -> p a b", a=4)
                        if t > 0:
                            tk_prev = tk - 1 if d == "f" else tk + 1
                            ps = gp.tile([128, 4, pb], fp32, tag=f"g{d}", name=f"g{d}")
                            h_prev = hT[d][:, tk_prev, :]
                            for j in range(4):
                                nc.tensor.matmul(
                                    ps[:, j, :],
                                    sb_whh[d][:, j * 128:(j + 1) * 128],
                                    h_prev,
                                    start=True, stop=True,
                                    skip_group_check=True,
                                )
                            nc.vector.tensor_add(
                                gs3,
                                ps[:],
                                xg[d][:, :, tk * pb:(tk + 1) * pb],
                            )
                        else:
                            nc.vector.tensor_copy(
                                gs3,
                                xg[d][:, :, tk * pb:(tk + 1) * pb],
                            )

                        a_ifo = step.tile([128, 3 * pb], fp32, tag=f"a{d}", name=f"a{d}")
                        nc.scalar.activation(a_ifo[:], gs[:, 0:3 * pb],
                                             AF.Sigmoid)
                        t_g = step.tile([128, pb], fp32, tag=f"tg{d}", name=f"tg{d}")
                        nc.scalar.activation(t_g[:], gs[:, 3 * pb:4 * pb],
                                             AF.Tanh)

                        m2 = step.tile([128, pb], fp32, tag=f"m2{d}", name=f"m2{d}")
                        nc.vector.tensor_mul(m2[:], a_ifo[:, 0:pb], t_g[:])
                        if t > 0:
                            m1 = step.tile([128, pb], fp32, tag=f"m1{d}", name=f"m1{d}")
                            nc.vector.tensor_mul(m1[:], a_ifo[:, pb:2 * pb],
                                                 c_prev[:])
                            nc.vector.tensor_add(c_new[:], m1[:], m2[:])
                        else:
                            nc.vector.tensor_copy(c_new[:], m2[:])

                        t_c = step.tile([128, pb], fp32, tag=f"tc{d}", name=f"tc{d}")
                        nc.scalar.activation(t_c[:], c_new[:], AF.Tanh)
                        nc.vector.tensor_mul(hT[d][:, tk, :],
                                             a_ifo[:, 2 * pb:3 * pb], t_c[:])

            # ---- phase 5: emissions into PSUM [(s16,b), blk, T] ----
            from contextlib import ExitStack
            _emctx = ExitStack()
            empool = _emctx.enter_context(
                tc.tile_pool(name="empool", bufs=1, space="PSUM"))
            em_ps = empool.tile([128, nblk, T], fp32, tag="emps", name="emps")
            for k in range(nblk):
                nc.tensor.matmul(
                    em_ps[:, k, :],
                    hT["f"][:].rearrange("p s b -> p (s b)")
                    [:, k * 128:(k + 1) * 128],
                    sb_wof[:],
                    start=True, stop=False, skip_group_check=True,
                )
                nc.tensor.matmul(
                    em_ps[:, k, :],
                    hT["b"][:].rearrange("p s b -> p (s b)")
                    [:, k * 128:(k + 1) * 128],
                    sb_wob[:],
                    start=False, stop=True, skip_group_check=True,
                )

            # stage PSUM -> SBUF, then reshuffle [(s16,b) part, blk, T]
            # -> em_sb [b part, s, T] with per-b DMAs
            em_stage = big.tile([128, nblk, T], fp32, tag="emstage",
                                name="emstage")
            half = (nblk // 2) * T
            nc.vector.tensor_copy(
                em_stage[:].rearrange("p a b -> p (a b)")[:, 0:half],
                em_ps[:].rearrange("p a b -> p (a b)")[:, 0:half])
            nc.scalar.copy(
                em_stage[:].rearrange("p a b -> p (a b)")[:, half:nblk * T],
                em_ps[:].rearrange("p a b -> p (a b)")[:, half:nblk * T])
            s16cnt = 128 // pb
            _emctx.close()
            pitch = nblk * T
            for s16 in range(s16cnt):
                src_ap = bass.AP(
                    em_stage[:].tensor,
                    em_stage[:].offset + s16 * pb * pitch,
                    [[pitch, pb], [T, nblk], [1, T]],
                )
                dst_ap = bass.AP(
                    em_sb[:].tensor,
                    em_sb[:].offset + s16 * T,
                    [[s_len * T, pb], [s16cnt * T, nblk], [1, T]],
                )
                nc.sync.dma_start(out=dst_ap, in_=src_ap)

            # ---- phase 6: CRF Viterbi forward ----
            nc.vector.tensor_add(score[:, 0, :], em_sb[:, 0, :],
                                 sb_start[:])
            for t in range(1, s_len):
                tmp = crf.tile([pb, T * T], fp32, tag="tmp", name="tmp")
                prev = score[:, t - 1, :].rearrange(
                    "p (o c) -> p o c", o=1).to_broadcast([pb, T, T])
                nc.vector.tensor_tensor(
                    out=tmp[:], in0=prev,
                    in1=sb_trep[:], op=ALU.add)
                mx = crf.tile([pb, T], fp32, tag="mx", name="mx")
                nc.vector.tensor_reduce(
                    out=mx[:],
                    in_=tmp[:].rearrange("p (c q) -> p c q", q=T),
                    axis=AX.X, op=ALU.max)
                nc.vector.tensor_add(score[:, t, :], mx[:],
                                     em_sb[:, t, :])

            nc.sync.dma_start(out=d_scores.ap(), in_=score[:])

    nc.compile()
    return nc


def _prep_host(inputs, dt_np):
    """Build per-core in_maps from full inputs."""
    x = np.asarray(inputs["x"])
    emb = np.asarray(inputs["emb"], dtype=np.float32)
    w_out = np.asarray(inputs["w_out"], dtype=np.float32)
    b_out = np.asarray(inputs["b_out"], dtype=np.float32)
    start = np.asarray(inputs["start"], dtype=np.float32)
    trans = np.asarray(inputs["trans"], dtype=np.float32)

    def perm_rows(w):
        chunks = [w[i * H:(i + 1) * H] for i in range(4)]
        return np.concatenate([chunks[i] for i in GATE_PERM], axis=0)

    shared = {"emb_w": emb.astype(dt_np)}
    for d, (wi, wh, bb) in (("f", ("w_ih_f", "w_hh_f", "b_f")),
                            ("b", ("w_ih_b", "w_hh_b", "b_b"))):
        wih = perm_rows(np.asarray(inputs[wi], dtype=np.float32))
        whh = perm_rows(np.asarray(inputs[wh], dtype=np.float32))
        bias = perm_rows(np.asarray(inputs[bb],
                                    dtype=np.float32).reshape(-1, 1))[:, 0]
        shared[f"wih_{d}"] = np.ascontiguousarray(wih.T).astype(dt_np)
        shared[f"whh_{d}"] = np.ascontiguousarray(whh.T).astype(dt_np)
        shared[f"biasmat_{d}"] = bias.reshape(4, 128).astype(dt_np)
    ind = np.zeros((4, 4, 128), dtype=np.float32)
    for j in range(4):
        ind[j, j, :] = 1.0
    shared["bias_ind"] = ind.reshape(4, 512).astype(dt_np)
    shared["wout_f"] = np.ascontiguousarray(w_out[:, :H].T).astype(dt_np)
    shared["wout_b"] = np.ascontiguousarray(w_out[:, H:].T).astype(dt_np)
    shared["ident"] = np.eye(128, dtype=np.float32).astype(dt_np)
    shared["start_t"] = np.tile((start + b_out)[None, :], (PB, 1)).astype(
        np.float32)
    trep = (trans + b_out[None, :]).T.reshape(-1)  # [(c,p)]
    shared["transrep"] = np.tile(trep[None, :], (PB, 1)).astype(np.float32)

    in_maps = []
    for k in range(NCORES):
        xc = x[k * PB:(k + 1) * PB]              # [pb, S]
        ids = np.ascontiguousarray(
            xc.T.reshape(-1).reshape(NBLK, 128).T).astype(np.int32)
        m = dict(shared)
        m["ids"] = ids
        in_maps.append(m)
    return in_maps


def _host_finalize(scores, trans, end):
    """scores [B, S, T] f32 -> (path int32 [B,S], best f32 [B])."""
    final = scores[:, -1, :] + end[None, :]
    last = np.argmax(final, axis=-1).astype(np.int32)
    best = final.max(axis=-1).astype(np.float32)
    bidx = np.arange(scores.shape[0])
    path = np.empty((scores.shape[0], scores.shape[1]), dtype=np.int32)
    path[:, -1] = last
    tag = last
    for t in range(scores.shape[1] - 1, 0, -1):
        val = scores[:, t - 1, :] + trans[:, tag].T   # [B, T(prev)]
        tag = np.argmax(val, axis=-1).astype(np.int32)
        path[:, t - 1] = tag
    return path, best


def _reference_np(inputs):
    """Exact numpy fallback (general mask)."""
    x = np.asarray(inputs["x"])
    mask = np.asarray(inputs["mask"])
    emb = np.asarray(inputs["emb"], np.float32)
    xt = emb[x].transpose(1, 0, 2)

    def lstm(xg, whh):
        h = np.zeros((xg.shape[1], whh.shape[1]), np.float32)
        c = np.zeros_like(h)
        hs = []
        for g_t in xg:
            g = g_t + h @ whh.T
            i, f, gg, o = np.split(g, 4, -1)
            sig = lambda z: 1.0 / (1.0 + np.exp(-z))
            c = sig(f) * c + sig(i) * np.tanh(gg)
            h = sig(o) * np.tanh(c)
            hs.append(h)
        return np.stack(hs)

    xg_f = xt @ np.asarray(inputs["w_ih_f"], np.float32).T + np.asarray(
        inputs["b_f"], np.float32)
    xg_b = xt[::-1] @ np.asarray(inputs["w_ih_b"], np.float32).T + np.asarray(
        inputs["b_b"], np.float32)
    h = np.concatenate([lstm(xg_f, np.asarray(inputs["w_hh_f"], np.float32)),
                        lstm(xg_b, np.asarray(inputs["w_hh_b"],
                                              np.float32))[::-1]], -1)
    em = h @ np.asarray(inputs["w_out"], np.float32).T + np.asarray(
        inputs["b_out"], np.float32)
    trans = np.asarray(inputs["trans"], np.float32)
    m = mask.T
    sc = np.asarray(inputs["start"], np.float32) + em[0]
    hist = []
    for t in range(1, em.shape[0]):
        tot = sc[:, :, None] + trans[None]
        best, idx = tot.max(1), tot.argmax(1).astype(np.int32)
        hist.append(idx)
        sc = np.where(m[t][:, None], best + em[t], sc)
    final = sc + np.asarray(inputs["end"], np.float32)
    last = np.argmax(final, -1).astype(np.int32)
    best = final.max(-1)
    tags = [last]
    tag = last
    bidx = np.arange(x.shape[0])
    for t in range(em.shape[0] - 2, -1, -1):
        prev = hist[t][bidx, tag]
        tag = np.where(m[t + 1], prev, tag)
        tags.append(tag)
    path = np.stack(tags[::-1], 1).astype(np.int32)
    return path, best.astype(np.float32)


def kernel(**inputs):
    mask = np.asarray(inputs["mask"])
    if not mask.all():
        return _reference_np(inputs)

    dt_mm = DT_MM
    key = (S, PB, dt_mm, "f16")
    if key not in _PROGRAM_CACHE:
        _PROGRAM_CACHE[key] = build_program(S, PB, dt_mm, "f16")
    nc = _PROGRAM_CACHE[key]

    from concourse.bass_utils import run_bass_kernel_spmd
    in_maps = _prep_host(inputs, _np_dt(dt_mm))
    res = run_bass_kernel_spmd(nc, in_maps, core_ids=list(range(NCORES)))
    global LAST_RESULT
    LAST_RESULT = res
    scores = np.concatenate([res.results[k]["scores"]
                             for k in range(NCORES)], axis=0)
    trans = np.asarray(inputs["trans"], np.float32)
    end = np.asarray(inputs["end"], np.float32)
    path, best = _host_finalize(scores, trans, end)
    return path, best
